# revision 63
# baseline (speedup 1.0000x reference)
"""GNN message-passing NodeBlock kernel for 8 Trainium2 NeuronCores.

Problem:
    agg_a = segment_sum(edata_a, conn_a[1], 100000)   # [N, 64]
    agg_b = segment_sum(edata_b, conn_b[1], 100000)   # [N, 64]
    out   = concat([agg_a, agg_b, vdata], 1) @ W + b  # [N, 128]

Sharding: edges are sharded BY RECEIVER RANGE -- core c owns nodes
[c*12544, (c+1)*12544) and receives exactly the edges targeting them, so each
core computes its slice of the aggregation completely locally; no collective.

Final design, ~126 us vs the 217 us hi-fp8/lo-bf16 baseline (measured on HW;
the chip shows two power states -- identical NEFFs measure ~126 us or ~149 us,
detectable via DVE tensor_tensor medians 1484 vs 1781 ns):
  * Edge features travel as plain bf16 (2 B/elem, rel err ~2^-9): ONE
    64-column stationary + ONE 64-column matmul per 128-edge tile instead of
    the baseline's two.  (A fp8 hi|lo 128-column-stationary variant measured
    WORSE: with only 64 matmul columns per 128-col LDWEIGHTS the PE array
    duty cycle drops to ~20% and the HAM activity monitor holds the PE at
    its cold 1.2 GHz clock for the whole kernel.)
  * Types a and b accumulate into one PSUM block (feat rows 0:64 / 64:128).
  * The one-hot scatter matrices are built on DVE in the tile-major layout
    (contiguous matmul rhs -- a column-strided rhs AP measured ~133ns/MM).
    To still hit the DVE 2x_1p perf mode (2 elem/cyc/lane; needs a 16-bit
    innermost stride-1 AP on EVERY operand, where plain broadcast APs fall
    to 1x), the rel comparand is host-duplicated into adjacent pairs
    (relx[2t]=relx[2t+1]=rel[t]) and read via a 4D AP with innermost [1,2],
    and the iota comparand is materialized tile-major as a constant
    [128, NHP*64] table so its reads collapse to contiguous.
  * vdata, W and the output travel in bf16 (host converts); bias stays f32.
  * Startup: the iota table is generated on the otherwise-idle GPSIMD engine
    while rel's head + block 0's edges (split per half) lead the sync DMA
    ring; edge blocks stream as two ~0.6 MB transfers each on the sync ring
    (keeps >=2 pending so the ~2 us HWDGE completion latency pipelines; a
    single 2.9 MB transfer or routing edges via the scalar/ACT ring both
    measured slower), with 9 edge buffers of DMA run-ahead.

SPMD: one program for all 8 cores.  Per-(core,window) tile counts differ, so
windows are sorted by (tiles_a, tiles_b) per core and the per-step tile count
is the max across cores (order statistics align, padding stays small).
Padding slots carry rel=-1 (matches no iota column); their lhsT rows are
whatever the DMA brought (harmless: their one-hot column is all zero).
"""
import numpy as np
import ml_dtypes

import concourse.bass as bass
import concourse.tile as tile
from concourse import mybir
from concourse.bass_utils import run_bass_kernel_spmd
from concourse.vector_clock import ScopedClock

BF16 = ml_dtypes.bfloat16
FP8 = ml_dtypes.float8_e4m3

N_NODES = 100000
N_EDGES = 800000
D_EDGE = 64
D_NODE = 128
D_OUT = 128
N_CORES = 8
WIN = 64                   # nodes per window
WPC = 196                  # windows per core
NPC = WIN * WPC            # nodes per core (12544)
NTOT = NPC * N_CORES       # padded node space (100352)
BLK_STEPS = 8              # windows per phase-2 block (8*64 = 512 cols)
N_BLKS = (WPC + BLK_STEPS - 1) // BLK_STEPS  # 25
OUT_CHUNK = 2              # blocks per outT store
DMA_BLKS = 1               # blocks per edge dma_start (~1.4 MB transfers)

# ---------------------------------------------------------------------------
# compat patches for this container's walrus build
# ---------------------------------------------------------------------------

_MAX_WAITS = 1


def _patched_drain_and_barrier(self, tick_clock, wait_clock):
    nc = self.nc
    probe = nc.sync.nop(nofuse=True, hint="tile_drain_wait0")
    wait_clock.add_sem_waits(
        probe.ins, ScopedClock({None: tick_clock.global_clock})
    )
    si = probe.ins.sync_info
    waits = list(si.on_wait) if si is not None and si.on_wait else []
    if len(waits) > _MAX_WAITS:
        si.on_wait = waits[:_MAX_WAITS]
        for k in range(_MAX_WAITS, len(waits), _MAX_WAITS):
            n = nc.sync.nop(nofuse=True, hint=f"tile_drain_wait{k}")
            n.ins.sync_info = mybir.SyncInfo(
                on_wait=waits[k : k + _MAX_WAITS], on_update=[]
            )
    drain_inst = nc.sync.drain()
    wait_clock.add_sem_waits(
        drain_inst.ins, ScopedClock({None: tick_clock.global_clock})
    )
    dsi = drain_inst.ins.sync_info
    if dsi is not None and dsi.on_wait and len(dsi.on_wait) > _MAX_WAITS:
        dsi.on_wait = []
    nc.all_engine_barrier()
    assert self.sems is not None
    popped = nc._tile_sem_poison_stack.pop()
    assert popped is self._sem_poison
    nc.clear_and_free_semaphores(list(self.sems.allocated().values()))
    nc.all_engine_barrier()


def _split_multi_waits(nc):
    """This walrus build accepts one sync-wait per TPB instruction; move
    extra waits onto preceding same-engine NOPs."""
    for fn in nc.m.functions:
        for blk in fn.blocks:
            out = []
            changed = False
            for inst in blk.instructions:
                si = inst.sync_info
                if si is not None and si.on_wait and len(si.on_wait) > 1:
                    waits = list(si.on_wait)
                    for j, w in enumerate(waits[:-1]):
                        nop = mybir.InstNoOp(
                            name=f"{inst.name}_xw{j}", ins=[], outs=[]
                        )
                        nop.engine = inst.engine
                        nop.sync_info = mybir.SyncInfo(
                            on_wait=[w], on_update=[]
                        )
                        out.append(nop)
                    si.on_wait = [waits[-1]]
                    changed = True
                out.append(inst)
            if changed:
                blk.instructions = out


def _install_ntff_hook_shim():
    import sys
    import types

    if "antenv.axon_hooks" in sys.modules:
        return
    mod = types.ModuleType("antenv.axon_hooks")
    _hook = [None]
    mod.set_axon_ntff_profile_hook = lambda h: _hook.__setitem__(0, h)
    mod.get_axon_ntff_profile_hook = lambda: _hook[0]
    sys.modules["antenv.axon_hooks"] = mod
    try:
        import antenv

        antenv.axon_hooks = mod
    except ImportError:
        pass
    try:
        from trn_agent_boot.trn_boot import _ntff_profile_via_ctypes

        mod.set_axon_ntff_profile_hook(
            _ntff_profile_via_ctypes("/opt/axon/libaxon_pjrt.so")
        )
    except Exception:
        pass


tile.TileContext._drain_and_barrier = _patched_drain_and_barrier
_install_ntff_hook_shim()

# ---------------------------------------------------------------------------
# host-side sharding / packing
# ---------------------------------------------------------------------------


def _schedule(cnt_a2, cnt_b2):
    """Shared-window schedule: per-core window perms + per-step (cross-core
    max) tile counts, block layout, and per-(block,half) rel offsets."""
    ta_all = np.ceil(cnt_a2 / 128).astype(np.int32)
    tb_all = np.ceil(cnt_b2 / 128).astype(np.int32)
    perms = np.argsort(-(ta_all * 100 + tb_all), axis=1, kind="stable")
    tiles_a = np.take_along_axis(ta_all, perms, 1)
    tiles_b = np.take_along_axis(tb_all, perms, 1)
    na_step = np.maximum(tiles_a.max(axis=0), 1).astype(np.int64)  # [WPC]
    nb_step = np.maximum(tiles_b.max(axis=0), 1).astype(np.int64)

    # per-block half sizes (edge tiles); rel slots == edge slots
    blk_na, blk_nb = [], []
    eoff_a = np.zeros(WPC, np.int64)   # edge-tile offset of step's a-tiles
    eoff_b = np.zeros(WPC, np.int64)
    blk_e0 = []                        # edge-tile offset of each block
    e = 0
    for j in range(N_BLKS):
        i0 = j * BLK_STEPS
        steps = min(BLK_STEPS, WPC - i0)
        na = int(na_step[i0 : i0 + steps].sum())
        nb = int(nb_step[i0 : i0 + steps].sum())
        blk_e0.append(e)
        blk_na.append(na)
        blk_nb.append(nb)
        o = e
        for i in range(i0, i0 + steps):
            eoff_a[i] = o
            o += na_step[i]
        for i in range(i0, i0 + steps):
            eoff_b[i] = o
            o += nb_step[i]
        e += na + nb
    T_e = int(e)
    nhp_max = max(max(blk_na), max(blk_nb))
    return dict(
        perms=perms, na_step=na_step, nb_step=nb_step,
        blk_na=blk_na, blk_nb=blk_nb, blk_e0=blk_e0,
        eoff_a=eoff_a, eoff_b=eoff_b,
        T_e=T_e, nhp_max=int(nhp_max),
    )


def _preprocess(vdata, edata_a, edata_b, conn_a, conn_b, W_mat, b_vec):
    recv_a = np.asarray(conn_a[1]).astype(np.int64)
    recv_b = np.asarray(conn_b[1]).astype(np.int64)

    def bin_type(recv):
        gwin = recv >> 6  # global 64-node window id (core = gwin // WPC)
        order = np.argsort(gwin, kind="stable")
        counts = np.bincount(gwin, minlength=WPC * N_CORES)
        starts = np.zeros(WPC * N_CORES + 1, dtype=np.int64)
        np.cumsum(counts, out=starts[1:])
        return order, counts.reshape(N_CORES, WPC), starts

    ids_a, cnt_a2, st_a = bin_type(recv_a)
    ids_b, cnt_b2, st_b = bin_type(recv_b)

    S = _schedule(cnt_a2, cnt_b2)
    perms = S["perms"]
    T_e = S["T_e"]

    e_a = np.asarray(edata_a).astype(BF16)
    e_b = np.asarray(edata_b).astype(BF16)

    vdata = np.asarray(vdata)
    vpad = np.zeros((NTOT, D_NODE), dtype=np.float32)
    vpad[:N_NODES] = vdata

    Wf = np.asarray(W_mat, dtype=np.float32)
    w2 = np.ascontiguousarray(
        np.concatenate([Wf[0:128], Wf[128:256]], axis=1).astype(BF16)
    )  # [128, 256] = [W_ab | W_v]
    bf = np.asarray(b_vec).astype(np.float32).reshape(D_OUT, 1)

    in_maps = []
    for c in range(N_CORES):
        slot_eid = np.full(T_e * 128, -1, dtype=np.int64)
        slot_is_a = np.zeros(T_e * 128, dtype=bool)
        rel = np.full(T_e * 128, -1.0, dtype=np.float32)  # [tile*128 slots]
        for i in range(WPC):
            w = perms[c][i]
            g = c * WPC + w
            for ids, starts, cnts2, eoff, is_a in (
                (ids_a, st_a, cnt_a2, S["eoff_a"], True),
                (ids_b, st_b, cnt_b2, S["eoff_b"], False),
            ):
                cnt = cnts2[c, w]
                if cnt == 0:
                    continue
                eids = ids[starts[g] : starts[g] + cnt]
                s0 = eoff[i] * 128
                slot_eid[s0 : s0 + cnt] = eids
                slot_is_a[s0 : s0 + cnt] = is_a
                rec = recv_a[eids] if is_a else recv_b[eids]
                rel[s0 : s0 + cnt] = (rec & (WIN - 1)).astype(np.float32)
        idx = np.maximum(slot_eid, 0)
        eh = np.where(slot_is_a[:, None], e_a[idx], e_b[idx])
        # pad rows left as-is (their one-hot column is zero)
        eh = np.ascontiguousarray(
            eh.reshape(T_e, 128, 64).transpose(1, 0, 2)
        )  # [slot, tile, feat] bf16
        # relx[p, 2t] = relx[p, 2t+1] = rel[p, t]  (pairs for DVE 2x packing)
        relT = rel.reshape(T_e, 128).T.astype(BF16)  # [128, T_e]
        relx = np.ascontiguousarray(
            np.repeat(relT, 2, axis=1)
        )  # [128, 2*T_e]
        base = c * NPC
        nodes = (
            base + (perms[c][:, None] * WIN + np.arange(WIN)[None, :]).reshape(-1)
        )
        vT = np.ascontiguousarray(vpad[nodes].T.astype(BF16))  # [128, NPC]
        in_maps.append(
            {"eh": eh, "rel": relx, "vT": vT, "w2": w2, "bd": bf}
        )

    sched = (
        tuple(int(x) for x in S["na_step"]),
        tuple(int(x) for x in S["nb_step"]),
    )
    return in_maps, sched, perms


# ---------------------------------------------------------------------------
# device kernel
# ---------------------------------------------------------------------------

_NC_CACHE = {}


def _build(sched):
    na_step, nb_step = sched
    # recompute the block layout directly from the step counts (must match
    # the host-side _schedule)
    na_step = np.asarray(na_step, dtype=np.int64)
    nb_step = np.asarray(nb_step, dtype=np.int64)
    blk_na, blk_nb, blk_e0 = [], [], []
    e = 0
    for j in range(N_BLKS):
        i0 = j * BLK_STEPS
        steps = min(BLK_STEPS, WPC - i0)
        na = int(na_step[i0 : i0 + steps].sum())
        nb = int(nb_step[i0 : i0 + steps].sum())
        blk_e0.append(e)
        blk_na.append(na)
        blk_nb.append(nb)
        e += na + nb
    T_e = e
    NHP = max(max(blk_na), max(blk_nb))
    max_blk = max(a + b for a, b in zip(blk_na, blk_nb))

    f32 = mybir.dt.float32
    bf16 = mybir.dt.bfloat16

    nc = bass.Bass(trn_type="TRN2")
    eh_d = nc.dram_tensor("eh", [128, T_e, 64], bf16, kind="ExternalInput")
    rel_d = nc.dram_tensor("rel", [128, 2 * T_e], bf16, kind="ExternalInput")
    vT_d = nc.dram_tensor("vT", [128, NPC], bf16, kind="ExternalInput")
    w2_d = nc.dram_tensor("w2", [128, 2 * D_OUT], bf16, kind="ExternalInput")
    b_d = nc.dram_tensor("bd", [D_OUT, 1], f32, kind="ExternalInput")
    outT_d = nc.dram_tensor("outT", [128, NPC], bf16, kind="ExternalOutput")
    # rel head covers the first two blocks' tiles (fast path to first sel)
    rel_head = 2 * (blk_e0[2] if N_BLKS > 2 else T_e)

    with tile.TileContext(nc) as tc:
        with (
            tc.tile_pool(name="consts", bufs=1) as cb,
            tc.tile_pool(name="xpool", bufs=4) as x0p,
            tc.tile_pool(name="edges", bufs=9) as ep,
            tc.tile_pool(name="sel", bufs=6) as sp,
            tc.tile_pool(name="out", bufs=3) as op,
            tc.tile_pool(name="psE", bufs=4, space="PSUM") as ppe,
            tc.tile_pool(name="psO", bufs=3, space="PSUM") as ppo,
        ):
            # fast path to the first sel build: iota generated on-device
            # (idle GPSIMD) while rel head rides the sync ring ahead of the
            # first edge block; rel tail and the rest on scalar
            # the small tail block (4 steps) is processed FIRST: its iota
            # prefix and rel range are tiny, so the pipeline starts ~4us
            # earlier than leading with the largest block
            j_first = N_BLKS - 1
            nh_first = max(blk_na[j_first], blk_nb[j_first])
            iota_sb = cb.tile([128, WIN * NHP], bf16, tag="iota")
            nc.gpsimd.iota(
                iota_sb[:, : WIN * nh_first].rearrange(
                    "p (t c) -> p t c", c=WIN
                ),
                pattern=[[0, nh_first], [1, WIN]],
                channel_multiplier=0,
                allow_small_or_imprecise_dtypes=True,
            )
            nc.gpsimd.iota(
                iota_sb[:, WIN * nh_first :].rearrange(
                    "p (t c) -> p t c", c=WIN
                ),
                pattern=[[0, NHP - nh_first], [1, WIN]],
                channel_multiplier=0,
                allow_small_or_imprecise_dtypes=True,
            )
            rel_sb = cb.tile([128, 2 * T_e], bf16, tag="rel")
            r_first = 2 * blk_e0[j_first]
            nc.sync.dma_start(rel_sb[:, r_first:], rel_d[:, r_first:])
            nc.sync.dma_start(rel_sb[:, :rel_head], rel_d[:, :rel_head])
            nc.scalar.dma_start(
                rel_sb[:, rel_head:r_first], rel_d[:, rel_head:r_first]
            )
            w2_sb = cb.tile([128, 2 * D_OUT], bf16, tag="w2")
            nc.scalar.dma_start(w2_sb[:], w2_d[:, :])
            b_sb = cb.tile([D_OUT, 1], f32, tag="b")
            nc.scalar.dma_start(b_sb[:], b_d[:, :])
            vt_sb = cb.tile([128, NPC], bf16, tag="vt")

            ot = None
            et2 = None
            max_grp = max(
                sum(blk_na[g] + blk_nb[g] for g in range(j0, min(j0 + DMA_BLKS, N_BLKS)))
                for j0 in range(0, N_BLKS, DMA_BLKS)
            )
            order = [N_BLKS - 1] + list(range(N_BLKS - 1))
            ot_groups = [[N_BLKS - 1]] + [
                list(range(a, min(a + OUT_CHUNK, N_BLKS - 1)))
                for a in range(0, N_BLKS - 1, OUT_CHUNK)
            ]
            grp_of = {j: g for g in ot_groups for j in g}
            for j in order:
                i0 = j * BLK_STEPS
                steps = min(BLK_STEPS, WPC - i0)
                cols_blk = steps * WIN
                na_b, nb_b = blk_na[j], blk_nb[j]
                n_blk = na_b + nb_b
                e0 = blk_e0[j]

                if j % DMA_BLKS == 0:
                    n_grp = sum(
                        blk_na[g] + blk_nb[g]
                        for g in range(j, min(j + DMA_BLKS, N_BLKS))
                    )
                    et2 = ep.tile([128, max_grp * 64], bf16, tag="et")
                    # per-half transfers keep >=2 pending on the ring, so
                    # the ~2us per-transfer completion latency pipelines
                    nc.sync.dma_start(
                        et2[:, : na_b * 64], eh_d[:, e0 : e0 + na_b, :]
                    )
                    nc.sync.dma_start(
                        et2[:, na_b * 64 : n_grp * 64],
                        eh_d[:, e0 + na_b : e0 + n_grp, :],
                    )
                    et_base = e0
                et_off = e0 - et_base  # tile offset of block j within et2
                # vT arrives in chunks woven between the early edge loads:
                # the first-processed (tail) block loads its own slice, then
                # blocks 0..4 cover the rest
                vt_last0 = (N_BLKS - 1) * BLK_STEPS * WIN
                if j == N_BLKS - 1:
                    nc.scalar.dma_start(
                        vt_sb[:, vt_last0:], vT_d[:, vt_last0:]
                    )
                elif j < 5:
                    c5 = vt_last0 // 5
                    vc0 = j * c5
                    vc1 = vt_last0 if j == 4 else (j + 1) * c5
                    nc.scalar.dma_start(vt_sb[:, vc0:vc1], vT_d[:, vc0:vc1])

                # one-hot build per half, tile-major layout: contiguous out
                # and iota; relx pairs via 4D AP with innermost [1,2] ->
                # every operand 16-bit innermost stride-1 -> DVE 2x mode
                sels = []
                for half, (r0, nh) in enumerate(
                    ((e0, na_b), (e0 + na_b, nb_b))
                ):
                    st = sp.tile([128, WIN * NHP], bf16, tag="sel")
                    in1 = rel_sb[:, 2 * r0 : 2 * (r0 + nh)].rearrange(
                        "p (t one cj) -> p t one cj", one=1, cj=2
                    ).broadcast_to([128, nh, WIN // 2, 2])
                    nc.vector.tensor_tensor(
                        out=st[:, : WIN * nh].rearrange(
                            "p (t ci cj) -> p t ci cj", ci=WIN // 2, cj=2
                        ),
                        in0=iota_sb[:, : WIN * nh].rearrange(
                            "p (t ci cj) -> p t ci cj", ci=WIN // 2, cj=2
                        ),
                        in1=in1,
                        op=mybir.AluOpType.is_equal,
                    )
                    sels.append(st)

                ps = ppe.tile([128, BLK_STEPS * WIN], f32, tag="ps")
                for half, n_stp in enumerate((na_step, nb_step)):
                    r0 = half * 64  # type a -> feat rows 0:64, b -> 64:128
                    st = sels[half]
                    tt = et_off + (blk_na[j] if half else 0)
                    t = 0
                    for stp in range(steps):
                        for k in range(n_stp[i0 + stp]):
                            nc.tensor.matmul(
                                out=ps[
                                    r0 : r0 + 64,
                                    stp * WIN : (stp + 1) * WIN,
                                ],
                                lhsT=et2[:, tt * 64 : (tt + 1) * 64],
                                rhs=st[:, t * WIN : (t + 1) * WIN],
                                start=(k == 0),
                                stop=(k == n_stp[i0 + stp] - 1),
                            )
                            t += 1
                            tt += 1

                x0 = x0p.tile([128, BLK_STEPS * WIN], bf16, tag="x0")
                nc.scalar.copy(x0[:, :cols_blk], ps[:, :cols_blk])

                po = ppo.tile([128, BLK_STEPS * WIN], f32, tag="po")
                nc.tensor.matmul(
                    out=po[:, :cols_blk], lhsT=w2_sb[:, 0:D_OUT],
                    rhs=x0[:, :cols_blk],
                    start=True, stop=False,
                )
                nc.tensor.matmul(
                    out=po[:, :cols_blk],
                    lhsT=w2_sb[:, D_OUT : 2 * D_OUT],
                    rhs=vt_sb[:, i0 * WIN : i0 * WIN + cols_blk],
                    start=False, stop=True,
                )
                grp = grp_of[j]
                jc = grp.index(j)
                if jc == 0:
                    ot = op.tile(
                        [128, OUT_CHUNK * BLK_STEPS * WIN], bf16, tag="ot"
                    )
                    chunk_col0 = i0 * WIN
                nc.scalar.activation(
                    out=ot[:, jc * BLK_STEPS * WIN : jc * BLK_STEPS * WIN + cols_blk],
                    in_=po[:, :cols_blk],
                    func=mybir.ActivationFunctionType.Identity,
                    bias=b_sb[:, 0:1],
                    scale=1.0,
                )
                if jc == len(grp) - 1:
                    chunk_cols = jc * BLK_STEPS * WIN + cols_blk
                    nc.scalar.dma_start(
                        outT_d[:, chunk_col0 : chunk_col0 + chunk_cols],
                        ot[:, :chunk_cols],
                    )
    _split_multi_waits(nc)
    return nc


# ---------------------------------------------------------------------------
# public entry point
# ---------------------------------------------------------------------------


def kernel(vdata, edata_a, edata_b, conn_a, conn_b, W, b, _trace=False):
    in_maps, sched, perms = _preprocess(
        vdata, edata_a, edata_b, conn_a, conn_b, W, b
    )
    nc = _NC_CACHE.get(sched)
    if nc is None:
        nc = _build(sched)
        _NC_CACHE[sched] = nc
    kwargs = {}
    if _trace:
        kwargs = dict(trace=True, trace_cores=[0])
    res = run_bass_kernel_spmd(
        nc, in_maps, core_ids=list(range(N_CORES)), **kwargs
    )

    out_full = np.empty((NTOT, D_OUT), dtype=np.float32)
    for c in range(N_CORES):
        outT = np.asarray(res.results[c]["outT"]).astype(np.float32)
        blocks = outT.reshape(D_OUT, WPC, WIN)
        base = c * NPC
        for i in range(WPC):
            w = perms[c][i]
            out_full[base + w * WIN : base + (w + 1) * WIN] = blocks[:, i, :].T
    out = out_full[:N_NODES]
    if _trace:
        return out, res
    return out
